# revision 1
# baseline (speedup 1.0000x reference)
"""Bi-attention kernel for Trainium2 (8 NeuronCores, data-parallel over batch).

Per-core computation (B=1 slice, Lc=512, Lq=64, D=256):
  score[i,j] = c_i.w_c + q_j.w_q + sum_d c[i,d] q[j,d] w_p[d] + b - 1e30*(1-mask[j])
  h = softmax_j(score);  U[i] = sum_j h[i,j] * (q_j.w_mem)
  u = softmax_i(max_j score);  H = sum_i u[i] * (c_i.w_in)
  G[i] = [ctx1[i], U[i], ctx1[i]*U[i], U[i]*H]

Sharding/layout choice: data-parallel over batch, one batch element per
core. Per-core inputs are laid out for the tensor engine during sharding:
context and question are shipped transposed (contraction dim D on
partitions), and all small params (att_w split, w_in, w_mem, att_b, qT)
ride in one packed [128, 139] tensor -> 7 DMAs total per core.

Device mapping:
  - score matmuls: per 128-row chunk, PSUM [128, 66] = scores | c.w_c |
    c.w_in; the per-column constants (q_j.w_q + b + mask) are added by a
    K=1 ones-row matmul into the same accumulation group.
  - row softmax: scores are O(10) so exp needs no max shift; ACT Exp with
    accum_out yields numerator-free denominator; the row max (needed for
    the second softmax's input m) runs in parallel on DVE.
  - softmax over i (partition dim): exp of per-chunk maxes, partition sums
    via ones-column matmul, scalar division, broadcast via ones-row matmul.
"""

import sys

for _p in ("/opt/trn_rl_repo", "/root/.axon_site/_ro/trn_rl_repo"):
    if _p not in sys.path:
        sys.path.append(_p)

import numpy as np

import concourse.bacc as bacc
import concourse.bass as bass
import concourse.tile as tile
from concourse import mybir
from concourse.bass_utils import run_bass_kernel_spmd

B, LC, LQ, D = 8, 512, 64, 256
NEG_BIG = 1e30
NCHUNK = LC // 128  # 4 chunks of 128 context rows
KD = D // 128  # 2 contraction chunks
F32 = mybir.dt.float32
I32 = mybir.dt.int32
AF = mybir.ActivationFunctionType
ALU = mybir.AluOpType
AX = mybir.AxisListType

# params tensor column layout (packed on host)
PC_WC = 0  # cols 0:2    w_c chunks
PC_WQ = 2  # cols 2:4    w_q chunks
PC_WP = 4  # cols 4:6    w_p chunks
PC_WIN = 6  # cols 6:8    w_in chunks
PC_WMEM = 8  # cols 8:10   w_mem chunks
PC_B = 10  # col 10      att_b at row 0
PC_QT = 11  # cols 11:139 qT chunks: [11+64k : 75+64k] = question.T chunk k
NPC = 11 + KD * LQ


def build_nc():
    nc = bacc.Bacc("TRN2", target_bir_lowering=False, debug=False)

    ctxt_d = nc.dram_tensor("contextT", [KD, 128, LC], F32, kind="ExternalInput")
    mask_d = nc.dram_tensor("mask", [1, LQ], I32, kind="ExternalInput")
    par_d = nc.dram_tensor("params", [128, NPC], F32, kind="ExternalInput")
    g_d = nc.dram_tensor("G", [LC, 4], F32, kind="ExternalOutput")

    with tile.TileContext(nc) as tc:
        with (
            tc.tile_pool(name="singles", bufs=1) as singles,
            tc.tile_pool(name="work", bufs=2) as work,
            tc.tile_pool(name="ps_sc", bufs=4, space="PSUM") as ps_sc,
            tc.tile_pool(name="ps_misc", bufs=1, space="PSUM") as ps_misc,
        ):
            # ---- params first on SP; context halves on Pool + SP ----
            par = singles.tile([128, NPC], F32)
            nc.sync.dma_start(out=par, in_=par_d[:, :])
            cT = singles.tile([128, KD, LC], F32)
            half = LC // 2
            nc.gpsimd.dma_start(out=cT[:, 0, 0:half], in_=ctxt_d[0, :, 0:half])
            nc.sync.dma_start(out=cT[:, 1, 0:half], in_=ctxt_d[1, :, 0:half])
            nc.gpsimd.dma_start(
                out=cT[:, 0, half:LC], in_=ctxt_d[0, :, half:LC]
            )
            nc.sync.dma_start(out=cT[:, 1, half:LC], in_=ctxt_d[1, :, half:LC])
            ones_row = singles.tile([1, 128], F32)
            nc.gpsimd.memset(ones_row, 1.0)
            ones_col = singles.tile([128, 1], F32)
            nc.gpsimd.memset(ones_col, 1.0)
            warm = singles.tile([1, 1], F32)
            nc.scalar.activation(warm, ones_row[0:1, 0:1], AF.Exp)
            mask_i = singles.tile([1, LQ], I32)
            nc.gpsimd.dma_start(out=mask_i, in_=mask_d[:, :])

            def qt(k):
                return par[:, PC_QT + LQ * k : PC_QT + LQ * (k + 1)]

            # rhsA_k [128, 66]: cols 0:64 = w_p * qT, col 64 = w_c, col 65 = w_in
            rhsA = []
            for k in range(KD):
                rhsA_k = singles.tile(
                    [128, LQ + 2], F32, tag=f"rhsA{k}", name=f"rhsA{k}"
                )
                nc.vector.tensor_scalar_mul(
                    rhsA_k[:, 0:LQ], qt(k), par[:, PC_WP + k : PC_WP + k + 1]
                )
                nc.vector.tensor_copy(
                    rhsA_k[:, LQ : LQ + 1], par[:, PC_WC + k : PC_WC + k + 1]
                )
                nc.vector.tensor_copy(
                    rhsA_k[:, LQ + 1 : LQ + 2], par[:, PC_WIN + k : PC_WIN + k + 1]
                )
                rhsA.append(rhsA_k)

            # ---- sq/q1 rows: [1, 64] each = w.T @ qT ----
            sq_ps = ps_misc.tile([1, LQ], F32, tag="early", name="sq_ps")
            for k in range(KD):
                nc.tensor.matmul(
                    sq_ps,
                    par[:, PC_WQ + k : PC_WQ + k + 1],
                    qt(k),
                    start=(k == 0),
                    stop=(k == KD - 1),
                )
            q1_ps = ps_misc.tile([1, LQ], F32, tag="earlyb", name="q1_ps")
            for k in range(KD):
                nc.tensor.matmul(
                    q1_ps,
                    par[:, PC_WMEM + k : PC_WMEM + k + 1],
                    qt(k),
                    start=(k == 0),
                    stop=(k == KD - 1),
                )

            # row_vec [1, 66]: cols j = sq[j] + b - 1e30*(1-mask[j]); 64,65 = 0
            row_vec = singles.tile([1, LQ + 2], F32)
            nc.gpsimd.memset(row_vec, 0.0)
            maskf = singles.tile([1, LQ], F32)
            nc.gpsimd.tensor_copy(maskf, mask_i)
            maskt = singles.tile([1, LQ], F32)
            nc.gpsimd.tensor_scalar(
                maskt, maskf, NEG_BIG, -NEG_BIG, op0=ALU.mult, op1=ALU.add
            )
            sqb = singles.tile([1, LQ], F32)
            nc.vector.tensor_scalar_add(sqb, sq_ps, par[0:1, PC_B : PC_B + 1])
            nc.vector.tensor_add(row_vec[0:1, 0:LQ], maskt, sqb)

            # q1 broadcast to all partitions: [128, 64] in PSUM
            q1row = singles.tile([1, LQ], F32)
            nc.vector.tensor_copy(q1row, q1_ps)
            q1bc_ps = ps_misc.tile([128, LQ], F32, tag="q1bc")
            nc.tensor.matmul(q1bc_ps, ones_row, q1row, start=True, stop=True)

            # ---- per-chunk: score matmuls + row softmax + U ----
            m_all = singles.tile([128, NCHUNK], F32)
            ctx1_all = singles.tile([128, NCHUNK], F32)
            g_all = singles.tile([128, NCHUNK, 4], F32)
            for c in range(NCHUNK):
                sc_ps = ps_sc.tile([128, LQ + 2], F32, tag="score", name=f"sc{c}")
                for k in range(KD):
                    nc.tensor.matmul(
                        sc_ps,
                        cT[:, k, 128 * c : 128 * (c + 1)],
                        rhsA[k],
                        start=(k == 0),
                        stop=False,
                    )
                nc.tensor.matmul(sc_ps, ones_row, row_vec, start=False, stop=True)

                t_ap = sc_ps[:, 0:LQ]
                rmax = work.tile([128, 1], F32, tag="rmax")
                nc.vector.tensor_reduce(rmax, t_ap, AX.X, ALU.max)
                nc.vector.tensor_add(m_all[:, c : c + 1], sc_ps[:, LQ : LQ + 1], rmax)
                # scores are O(10): exp is fp32-safe without max shift
                e_t = work.tile([128, LQ], F32, tag="e")
                den = work.tile([128, 1], F32, tag="den")
                nc.scalar.activation(e_t, t_ap, AF.Exp, accum_out=den)
                prod = work.tile([128, LQ], F32, tag="prod")
                num = work.tile([128, 1], F32, tag="num")
                nc.vector.tensor_mul(prod, e_t, q1bc_ps)
                nc.vector.reduce_sum(num, prod, axis=AX.X, op=ALU.add)
                rden = work.tile([128, 1], F32, tag="rden")
                nc.vector.reciprocal(rden, den)
                nc.vector.tensor_mul(g_all[:, c, 1:2], num, rden)  # U
                nc.vector.tensor_copy(ctx1_all[:, c : c + 1], sc_ps[:, LQ + 1 : LQ + 2])
                nc.vector.tensor_copy(g_all[:, c, 0:1], sc_ps[:, LQ + 1 : LQ + 2])
                nc.gpsimd.tensor_mul(
                    g_all[:, c, 2:3], ctx1_all[:, c : c + 1], g_all[:, c, 1:2]
                )

            # ---- u_aware softmax over i (512 values) + H ----
            exu = singles.tile([128, 2 * NCHUNK], F32)
            nc.scalar.activation(exu[:, 0:NCHUNK], m_all, AF.Exp)
            nc.gpsimd.tensor_mul(
                exu[:, NCHUNK : 2 * NCHUNK], exu[:, 0:NCHUNK], ctx1_all
            )
            hsum_ps = ps_misc.tile([1, 2 * NCHUNK], F32, tag="late")
            nc.tensor.matmul(hsum_ps, ones_col, exu, start=True, stop=True)
            dn = singles.tile([1, 2], F32)
            nc.vector.tensor_reduce(
                dn.rearrange("o (c f) -> o c f", c=2),
                hsum_ps[0:1, :].rearrange("o (c f) -> o c f", c=2),
                AX.X,
                ALU.add,
            )
            rden_u = singles.tile([1, 1], F32)
            nc.vector.reciprocal(rden_u, dn[0:1, 0:1])
            h_sb = singles.tile([1, 1], F32)
            nc.vector.tensor_mul(h_sb, dn[0:1, 1:2], rden_u)
            hbc_ps = ps_misc.tile([128, 1], F32, tag="late", name="hbc_ps")
            nc.tensor.matmul(hbc_ps, ones_row, h_sb, start=True, stop=True)

            nc.vector.tensor_scalar_mul(
                g_all[:, :, 3:4].rearrange("q c o -> q (c o)"),
                g_all[:, :, 1:2].rearrange("q c o -> q (c o)"),
                hbc_ps,
            )
            nc.sync.dma_start(
                out=g_d.rearrange("(c p) g -> p c g", p=128), in_=g_all
            )

    nc.finalize()
    return nc


_NC = None


def _get_nc():
    global _NC
    if _NC is None:
        _NC = build_nc()
    return _NC


def pack_params(att_w, att_b, w_in, w_mem, question_b):
    par = np.zeros((128, NPC), np.float32)
    par[:, PC_WC : PC_WC + 2] = att_w[0:256].reshape(2, 128).T
    par[:, PC_WQ : PC_WQ + 2] = att_w[256:512].reshape(2, 128).T
    par[:, PC_WP : PC_WP + 2] = att_w[512:768].reshape(2, 128).T
    par[:, PC_WIN : PC_WIN + 2] = w_in.reshape(2, 128).T
    par[:, PC_WMEM : PC_WMEM + 2] = w_mem.reshape(2, 128).T
    par[0, PC_B] = att_b[0]
    qt = question_b.T.reshape(KD, 128, LQ)  # [d, j] split into chunks
    for k in range(KD):
        par[:, PC_QT + LQ * k : PC_QT + LQ * (k + 1)] = qt[k]
    return par


def make_in_maps(context, question, mask, att_w, att_b, w_in, w_mem):
    context = np.asarray(context, np.float32)
    question = np.asarray(question, np.float32)
    mask = np.asarray(mask, np.int32)
    att_w = np.asarray(att_w, np.float32)
    att_b = np.asarray(att_b, np.float32)
    w_in = np.asarray(w_in, np.float32)
    w_mem = np.asarray(w_mem, np.float32)
    maps = []
    for b in range(B):
        ctxt = np.ascontiguousarray(context[b].T).reshape(KD, 128, LC)
        maps.append(
            {
                "contextT": ctxt,
                "mask": mask[b][None, :],
                "params": pack_params(att_w, att_b, w_in, w_mem, question[b]),
            }
        )
    return maps


def kernel(context, question, mask, att_w, att_b, w_in, w_mem):
    nc = _get_nc()
    in_maps = make_in_maps(context, question, mask, att_w, att_b, w_in, w_mem)
    res = run_bass_kernel_spmd(nc, in_maps, core_ids=list(range(B)))
    return np.stack([res.results[c]["G"] for c in range(B)], axis=0)



# revision 12
# speedup vs baseline: 1.4885x; 1.4885x over previous
"""Bi-attention kernel for Trainium2 (8 NeuronCores, data-parallel over batch).

Per-core computation (B=1 slice, Lc=512, Lq=64, D=256):
  score[i,j] = sum_d c[i,d] q[j,d] w_p[d] + rv_j,  rv_j = q_j.w_q + b - 1e30*(1-mask_j)
  h = softmax_j(score);  U[i] = sum_j h[i,j] * (q_j.w_mem)
  m_i = max_j score[i,j] + c_i.w_c;  u = softmax_i(m);  H = sum_i u[i]*ctx1[i]
  ctx1 = c.w_in;  G[i] = [ctx1[i], U[i], ctx1[i]*U[i], U[i]*H]
(The c_i.w_c term is constant per row i, so it cancels inside softmax_j and
only needs to be added to the row max for the u softmax.)

Schedule (tuned against the CoreSim legacy cost model):
  - 3 input DMAs (two contextT halves + one packed bf16 param tensor) are
    issued at t~100 on three different engine queues (SP/Act/DVE); each DMA
    is 500ns of engine time + ~1717ns fixed latency -> inputs land ~2317ns.
  - All matmuls run in bf16 (1 cycle/row vs 4 for fp32). Scores accumulate
    into two half PSUM tiles [128, 2*66] so the first half can flow into
    softmax while PE finishes the second. The per-column bias rv is added
    by a K=1 ones-row matmul per chunk, which also stops the accumulation.
  - Exp runs in 2 halves on Act; row-max/row-sum and all small elementwise
    ops run on Pool (cost = free_size only, no fixed overhead); the U
    numerator uses DVE tensor_tensor_reduce.
  - The u-softmax partition sum AND broadcast happen in ONE all-ones
    [128,128] matmul; the rest of the tail stays on Pool (no sem ping-pong).
  - Output is a single DMA issued from the (idle) SP queue.
"""

import sys

for _p in ("/opt/trn_rl_repo", "/root/.axon_site/_ro/trn_rl_repo"):
    if _p not in sys.path:
        sys.path.append(_p)

import numpy as np
import ml_dtypes

import concourse.bacc as bacc
import concourse.bass as bass
import concourse.bass_isa as bass_isa
import concourse.tile as tile
from concourse import mybir
from concourse.bass_utils import run_bass_kernel_spmd

B, LC, LQ, D = 8, 512, 64, 256
NEG_BIG = 1e30
NCHUNK = LC // 128  # 4 chunks of 128 context rows
KD = D // 128  # 2 contraction chunks
GW = LQ + 2  # psum group width: 64 scores | c.w_c | c.w_in
F32 = mybir.dt.float32
BF16 = mybir.dt.bfloat16
AF = mybir.ActivationFunctionType
ALU = mybir.AluOpType
AX = mybir.AxisListType
BF = ml_dtypes.bfloat16

# packed bf16 params column layout
PC_QT = 0  # cols 0:128    qT chunks (64 cols each)
PC_WQM = 128  # cols 128:132  [w_q | w_mem] per k chunk
PC_WP = 132  # cols 132:134  w_p chunks
PC_WC = 134  # cols 134:136  w_c chunks
PC_WIN = 136  # cols 136:138  w_in chunks
PC_B = 138  # col 138       att_b at row 0
PC_MK = 139  # cols 139:203  maskt = -1e30*(1-mask) at row 0
NPB = PC_MK + LQ


def build_nc():
    nc = bacc.Bacc("TRN2", target_bir_lowering=False, debug=False)

    ctxb_d = nc.dram_tensor("ctxb", [128, KD, LC], BF16, kind="ExternalInput")
    parb_d = nc.dram_tensor("parb", [128, NPB], BF16, kind="ExternalInput")
    g_d = nc.dram_tensor("G", [LC, 4], F32, kind="ExternalOutput")

    with tile.TileContext(nc) as tc:
        with (
            tc.tile_pool(name="singles", bufs=1) as singles,
            tc.tile_pool(name="ps_sc", bufs=2, space="PSUM") as ps_sc,
            tc.tile_pool(name="ps_sq", bufs=1, space="PSUM") as ps_sq,
        ):
            # ---- input DMAs: one per DMA-capable queue, issued immediately.
            # parb gates everything -> SP (fastest); the k=1 context half is
            # needed last -> Pool (SWDGE, +166ns extra latency is harmless).
            parb = singles.tile([128, NPB], BF16)
            nc.sync.dma_start(out=parb, in_=parb_d[:, :])
            cb = singles.tile([128, KD, LC], BF16)
            nc.scalar.dma_start(out=cb[:, 0, :], in_=ctxb_d[:, 0, :])
            nc.gpsimd.dma_start(out=cb[:, 1, :], in_=ctxb_d[:, 1, :])

            # ---- constants (Pool) + act table warm (Act) ----
            ones1 = singles.tile([1, 128], BF16)
            nc.gpsimd.memset(ones1, 1.0)
            rv66 = singles.tile([1, GW], BF16)
            nc.gpsimd.memset(rv66, 0.0)
            warm1 = singles.tile([1, 1], F32)
            nc.gpsimd.memset(warm1, 0.0)
            warmo = singles.tile([1, 1], F32)
            nc.scalar.activation(warmo, warm1, AF.Exp)

            def qt(k):
                return parb[:, PC_QT + LQ * k : PC_QT + LQ * (k + 1)]

            # ---- sq/q1 rows: [2, 64] = [w_q | w_mem].T @ qT (PE first) ----
            sqq1 = ps_sq.tile([2, LQ], F32, tag="sqq1")
            for k in range(KD):
                nc.tensor.matmul(
                    sqq1,
                    parb[:, PC_WQM + 2 * k : PC_WQM + 2 * k + 2],
                    qt(k),
                    start=(k == 0),
                    stop=(k == KD - 1),
                )

            # ---- rhsA_k [128, 66] = [w_p*qT | w_c | w_in] (Pool) ----
            # scalar-ptr operands must be fp32: convert w_p / b first
            wpf = singles.tile([128, KD], F32)
            nc.gpsimd.tensor_copy(wpf, parb[:, PC_WP : PC_WP + KD])
            bf1 = singles.tile([1, 1], F32)
            nc.gpsimd.tensor_copy(bf1, parb[0:1, PC_B : PC_B + 1])
            rhsA = []
            for k in range(KD):
                rhsA_k = singles.tile([128, GW], BF16, tag=f"rhsA{k}", name=f"rhsA{k}")
                nc.gpsimd.tensor_scalar_mul(
                    rhsA_k[:, 0:LQ], qt(k), wpf[:, k : k + 1]
                )
                nc.gpsimd.tensor_copy(
                    rhsA_k[:, LQ : LQ + 1], parb[:, PC_WC + k : PC_WC + k + 1]
                )
                nc.gpsimd.tensor_copy(
                    rhsA_k[:, LQ + 1 : LQ + 2], parb[:, PC_WIN + k : PC_WIN + k + 1]
                )
                rhsA.append(rhsA_k)

            # rv = sq + b + maskt (bf16, Pool); q1 row copy for broadcast
            nc.gpsimd.scalar_tensor_tensor(
                rv66[0:1, 0:LQ],
                sqq1[0:1, :],
                bf1,
                parb[0:1, PC_MK : PC_MK + LQ],
                op0=ALU.add,
                op1=ALU.add,
            )
            q1row = singles.tile([1, LQ], BF16)
            nc.gpsimd.tensor_copy(q1row, sqq1[1:2, :])

            # ---- scores: two half PSUM tiles [128, 2*66] ----
            # Only one accumulation group may be open per tile, so interleave
            # the two tiles: open c0 (S0) and c2 (S1) with k-matmuls while rv
            # is still in flight, close c0 the moment rv lands, then run
            # c1 / close it (unblocks exp half 0), then c2b / c3 (half 1).
            Sh = []
            for h in range(2):
                S = ps_sc.tile([128, 2 * GW], F32, tag="score", name=f"S{h}")
                Sh.append(S.rearrange("p (c w) -> p c w", c=2))

            def kmm(c, k):
                h, cc = divmod(c, 2)
                nc.tensor.matmul(
                    Sh[h][:, cc, :],
                    cb[:, k, 128 * c : 128 * (c + 1)],
                    rhsA[k],
                    start=(k == 0),
                    stop=False,
                )

            def bmm(c):
                h, cc = divmod(c, 2)
                nc.tensor.matmul(Sh[h][:, cc, :], ones1, rv66, start=False, stop=True)

            kmm(0, 0)
            kmm(0, 1)
            kmm(2, 0)
            kmm(2, 1)
            bmm(0)
            kmm(1, 0)
            kmm(1, 1)
            bmm(1)
            bmm(2)
            kmm(3, 0)
            kmm(3, 1)
            bmm(3)

            # q1 broadcast to all partitions (Pool, no PE round-trip)
            q1bc = singles.tile([128, LQ], BF16)
            nc.gpsimd.partition_broadcast(q1bc, q1row)

            # c.w_c / c.w_in columns gathered per half as soon as chunks stop
            def scol(h, j):
                return Sh[h][:, :, j : j + 1].rearrange("p c o -> p (c o)")

            G = singles.tile([128, NCHUNK, 4], F32)

            def gcol(j):
                return G[:, :, j : j + 1].rearrange("p c o -> p (c o)")

            cwc4 = singles.tile([128, NCHUNK], F32)
            for h in range(2):
                nc.gpsimd.tensor_copy(cwc4[:, 2 * h : 2 * h + 2], scol(h, LQ))
                nc.gpsimd.tensor_copy(gcol(0)[:, 2 * h : 2 * h + 2], scol(h, LQ + 1))

            # ---- softmax_j pipeline over halves ----
            # E = exp(scores) on Act; row-sum (den) and row-max (M) of E via
            # Pool binary trees; U numerator via DVE tensor_tensor_reduce.
            E = singles.tile([128, NCHUNK, LQ], BF16)
            den4 = singles.tile([128, NCHUNK], F32)
            M4 = singles.tile([128, NCHUNK], BF16)
            num4 = singles.tile([128, NCHUNK], F32)
            prods = singles.tile([128, LQ], BF16)
            dscr = singles.tile([128, 2, 32], F32)
            mscr = singles.tile([128, 2, 32], BF16)

            def pool_tree(dst2, src3, scr, op):
                # src3: [128, 2, 64] -> dst2: [128, 2] via contiguous halving
                nc.gpsimd.tensor_tensor(scr, src3[:, :, 0:32], src3[:, :, 32:64], op)
                w = 16
                while w >= 1:
                    a = scr[:, :, 0:w]
                    if w == 1:
                        a = dst2.rearrange("p (c o) -> p c o", o=1)
                    nc.gpsimd.tensor_tensor(
                        a, scr[:, :, 0:w], scr[:, :, w : 2 * w], op
                    )
                    w //= 2

            for h in range(2):
                sc_h = Sh[h][:, :, 0:LQ]
                nc.scalar.activation(E[:, 2 * h : 2 * h + 2, :], sc_h, AF.Exp)
            for h in range(2):
                Eh = E[:, 2 * h : 2 * h + 2, :]
                pool_tree(den4[:, 2 * h : 2 * h + 2], Eh, dscr, ALU.add)
                pool_tree(M4[:, 2 * h : 2 * h + 2], Eh, mscr, ALU.max)
            for c in range(NCHUNK):
                nc.vector.tensor_tensor_reduce(
                    out=prods,
                    in0=E[:, c, :],
                    in1=q1bc,
                    scale=1.0,
                    scalar=0.0,
                    op0=ALU.mult,
                    op1=ALU.add,
                    accum_out=num4[:, c : c + 1],
                )

            # ---- u softmax over i, entirely on Pool after one Act exp:
            # e^{m_i} = max_j E * e^{c_i.w_c}; sums via partition_all_reduce.
            ecwc4 = singles.tile([128, NCHUNK], F32)
            nc.scalar.activation(ecwc4, cwc4, AF.Exp)
            em4 = singles.tile([128, NCHUNK], F32)
            emc4 = singles.tile([128, NCHUNK], F32)
            em2 = singles.tile([128, 2], F32)
            em2a = singles.tile([128, 2], F32)
            t2 = singles.tile([128, 2], F32)
            nc.gpsimd.tensor_tensor(em4, M4, ecwc4, ALU.mult)
            nc.gpsimd.tensor_tensor(emc4, em4, gcol(0), ALU.mult)
            nc.gpsimd.tensor_tensor(t2, em4[:, 0:2], em4[:, 2:4], ALU.add)
            nc.gpsimd.tensor_tensor(em2[:, 0:1], t2[:, 0:1], t2[:, 1:2], ALU.add)
            nc.gpsimd.tensor_tensor(t2, emc4[:, 0:2], emc4[:, 2:4], ALU.add)
            nc.gpsimd.tensor_tensor(em2[:, 1:2], t2[:, 0:1], t2[:, 1:2], ALU.add)
            nc.gpsimd.partition_all_reduce(em2a, em2, 128, bass_isa.ReduceOp.add)
            h1 = singles.tile([128, 1], F32)
            nc.gpsimd.tensor_tensor(h1, em2a[:, 1:2], em2a[:, 0:1], ALU.divide)

            nc.gpsimd.tensor_tensor(gcol(1), num4, den4, ALU.divide)  # U
            nc.gpsimd.tensor_tensor(gcol(2), gcol(0), gcol(1), ALU.mult)
            nc.gpsimd.tensor_scalar_mul(gcol(3), gcol(1), h1)

            nc.sync.dma_start(out=g_d.rearrange("(c p) g -> p c g", p=128), in_=G)

    nc.finalize()
    return nc


_NC = None


def _get_nc():
    global _NC
    if _NC is None:
        _NC = build_nc()
    return _NC


def pack_params(att_w, att_b, w_in, w_mem, mask_b, question_b):
    par = np.zeros((128, NPB), np.float32)
    qt = question_b.T.reshape(KD, 128, LQ)
    for k in range(KD):
        par[:, PC_QT + LQ * k : PC_QT + LQ * (k + 1)] = qt[k]
        par[:, PC_WQM + 2 * k] = att_w[256 + 128 * k : 256 + 128 * (k + 1)]
        par[:, PC_WQM + 2 * k + 1] = w_mem[128 * k : 128 * (k + 1)]
        par[:, PC_WP + k] = att_w[512 + 128 * k : 512 + 128 * (k + 1)]
        par[:, PC_WC + k] = att_w[128 * k : 128 * (k + 1)]
        par[:, PC_WIN + k] = w_in[128 * k : 128 * (k + 1)]
    par[0, PC_B] = att_b[0]
    par[0, PC_MK : PC_MK + LQ] = -NEG_BIG * (1.0 - mask_b.astype(np.float32))
    return par.astype(BF)


def make_in_maps(context, question, mask, att_w, att_b, w_in, w_mem):
    context = np.asarray(context, np.float32)
    question = np.asarray(question, np.float32)
    mask = np.asarray(mask, np.int32)
    att_w = np.asarray(att_w, np.float32)
    att_b = np.asarray(att_b, np.float32)
    w_in = np.asarray(w_in, np.float32)
    w_mem = np.asarray(w_mem, np.float32)
    maps = []
    for b in range(B):
        # ctxb[p, k, i] = context[b, i, 128*k + p]
        ctxb = np.ascontiguousarray(
            np.moveaxis(context[b].T.reshape(KD, 128, LC), 0, 1)
        ).astype(BF)
        maps.append(
            {
                "ctxb": ctxb,
                "parb": pack_params(att_w, att_b, w_in, w_mem, mask[b], question[b]),
            }
        )
    return maps


def kernel(context, question, mask, att_w, att_b, w_in, w_mem):
    nc = _get_nc()
    in_maps = make_in_maps(context, question, mask, att_w, att_b, w_in, w_mem)
    res = run_bass_kernel_spmd(nc, in_maps, core_ids=list(range(B)))
    return np.stack([res.results[c]["G"] for c in range(B)], axis=0)
